# revision 5
# baseline (speedup 1.0000x reference)
"""Cross-attention Trainium2 kernel (Bass/Tile), 8-core SPMD.

Sharding: 8 cores = 2 (batch) x 4 (head groups of 3 heads).
Each core computes, for its (b, g):
    q^T = Wq_g @ x_b^T          [192, 2048]  (+bq)
    k^T = Wk_g @ y_b^T          [192, 2048]  (+bk)
    v   = y_b @ Wv_g^T          [2048, 192]
    per head: S^T = k_h q_h^T   [2048(m), 2048(l)] tiles in PSUM
              P^T = exp(S^T/8)  (softmax numerator, bf16)
              O^T = v_h^T P^T   (PSUM accumulated over m)
              den = 1^T P^T, O_n^T = O^T * (1/den)  (broadcast via PE)
    partial^T = Wp_g^T^T @ O_n^T  [768, 2048]  -> DRAM fp32
Host: out[b] = sum_g partial_g^T.T + Wp @ bv + bp.

Engine budget per slot of 16 m-blocks (ACT is the floor at ~17.1us):
exp on ACT; den accumulated on 2 Vector chains + 1 GpSimd chain;
PSUM evacuation on Vector (steady) / Scalar (ramp, ACT idle);
K/Q head-2 dup copies on GpSimd (SBUF->SBUF).
"""

import os
import sys
from contextlib import ExitStack

import numpy as np

for _p in ("/opt/trn_rl_repo", "/root/.axon_site/_ro/trn_rl_repo"):
    if os.path.isdir(_p) and _p not in sys.path:
        sys.path.insert(0, _p)

try:  # make trace=True harmless when the env lacks the NTFF hook module
    import antenv.axon_hooks  # noqa: F401
except Exception:
    import types

    _stub = types.ModuleType("antenv.axon_hooks")
    _stub.get_axon_ntff_profile_hook = lambda: None
    _stub.set_axon_ntff_profile_hook = lambda hook: None
    sys.modules["antenv.axon_hooks"] = _stub

import concourse.bass as bass
import concourse.tile as tile
from concourse import bacc as bacc_mod
from concourse import mybir
from concourse.bass_utils import run_bass_kernel_spmd
from ml_dtypes import bfloat16

F32 = mybir.dt.float32
BF16 = mybir.dt.bfloat16
EXP = mybir.ActivationFunctionType.Exp

B = 2
L = 2048          # query length (also key length)
D = 768
HD = 64           # head dim
HPC = 3           # heads per core
GW = HPC * HD     # 192: head-group width
KT = D // 128     # 6 contraction tiles for V projection
KTA = KT + 1      # 7 tiles for Q/K: 7th carries the bias row (exact bias fold)
DA = KTA * 128    # 896: augmented contraction depth
NLC = L // 512    # 4 l-chunks
NM = L // 128     # 16 m-tiles
SCALE = 1.0 / 8.0  # hd ** -0.5


def _build_program(nkt_qk=KTA):
    nc = bacc_mod.Bacc()

    da = nkt_qk * 128
    xT = nc.dram_tensor("xT", [da, L], BF16, kind="ExternalInput")[:, :]
    yT = nc.dram_tensor("yT", [da, L], BF16, kind="ExternalInput")[:, :]
    wqT = nc.dram_tensor("wqT", [da, GW], BF16, kind="ExternalInput")[:, :]
    wkT = nc.dram_tensor("wkT", [da, GW], BF16, kind="ExternalInput")[:, :]
    wvT = nc.dram_tensor("wvT", [D, GW], BF16, kind="ExternalInput")[:, :]
    wpT = nc.dram_tensor("wpT", [GW, D], BF16, kind="ExternalInput")[:, :]
    pT = nc.dram_tensor("pT", [D, L], F32, kind="ExternalOutput")[:, :]

    with tile.TileContext(nc) as tc, ExitStack() as ctx:
        persist = ctx.enter_context(tc.tile_pool(name="persist", bufs=1))
        spool = ctx.enter_context(tc.tile_pool(name="spool", bufs=2, space="PSUM"))
        opool = ctx.enter_context(tc.tile_pool(name="opool", bufs=1, space="PSUM"))
        projpool = ctx.enter_context(tc.tile_pool(name="projpool", bufs=3, space="PSUM"))
        ptpool = ctx.enter_context(tc.tile_pool(name="ptpool", bufs=8))
        accpool = ctx.enter_context(tc.tile_pool(name="accpool", bufs=2))
        ospool = ctx.enter_context(tc.tile_pool(name="ospool", bufs=2))
        rpool = ctx.enter_context(tc.tile_pool(name="rpool", bufs=2))
        bcpool = ctx.enter_context(tc.tile_pool(name="bcpool", bufs=2))

        # ---------------- persistent SBUF tensors --------------------------
        xT_sb = persist.tile([128, nkt_qk, L], BF16, tag="xT")
        yT_sb = persist.tile([128, nkt_qk, L], BF16, tag="yT")
        wq_sb = persist.tile([128, nkt_qk, GW], BF16, tag="wq")
        wk_sb = persist.tile([128, nkt_qk, GW], BF16, tag="wk")
        wv_sb = persist.tile([128, KT, GW], BF16, tag="wv")
        wp_a = persist.tile([128, D], BF16, tag="wpa")
        wp_b = persist.tile([64, D], BF16, tag="wpb")
        garb = persist.tile([128, 256], BF16, tag="garb")
        ones_col = persist.tile([128, 1], BF16, tag="onesc")
        dex = persist.tile([1, 8], F32, tag="dex")

        qT_p = persist.tile([128, L], BF16, tag="qTp")   # heads 0,1 stacked
        qT_2 = persist.tile([128, L], BF16, tag="qT2")   # head 2, dup halves
        kT_p = persist.tile([128, L], BF16, tag="kTp")
        kT_2 = persist.tile([128, L], BF16, tag="kT2")   # head 2, dup halves
        v_sb = persist.tile([128, NM, GW], BF16, tag="v")
        on_p = persist.tile([128, L], BF16, tag="onp")   # normalized O^T heads 0,1
        on_2 = persist.tile([64, L], BF16, tag="on2")    # head 2

        # ---------------- warmup: PE p-state + exp table preload -----------
        nc.vector.memset(garb, 0.0)
        nc.vector.memset(ones_col, 1.0)
        nc.vector.memset(dex, 0.0)
        nc.scalar.activation(dex, dex, EXP)  # pull exp table load to t=0
        wps = projpool.tile([1, 256], F32, tag="proj")
        for _ in range(12):
            nc.tensor.matmul(wps, ones_col, garb, start=True, stop=True)

        # ---------------- input DMA (first-needed first) --------------------
        xT_r = xT.rearrange("(kt p) l -> p kt l", p=128)
        yT_r = yT.rearrange("(kt p) l -> p kt l", p=128)
        wk_r = wkT.rearrange("(kt p) g -> p kt g", p=128)
        wq_r = wqT.rearrange("(kt p) g -> p kt g", p=128)
        wv_r = wvT.rearrange("(kt p) g -> p kt g", p=128)
        q0, q1 = slice(0, 512), slice(512, 1024)
        h1 = slice(L // 2, L)
        nc.sync.dma_start(out=wk_sb, in_=wk_r)
        for kt in range(nkt_qk):
            nc.sync.dma_start(out=yT_sb[:, kt, q0], in_=yT_r[:, kt, q0])
        nc.sync.dma_start(out=wv_sb, in_=wv_r)
        nc.sync.dma_start(out=wq_sb, in_=wq_r)
        for kt in range(nkt_qk):
            nc.sync.dma_start(out=yT_sb[:, kt, q1], in_=yT_r[:, kt, q1])
        for kt in range(nkt_qk):
            nc.sync.dma_start(out=xT_sb[:, kt, 0:1024], in_=xT_r[:, kt, 0:1024])
        for kt in range(nkt_qk):
            nc.sync.dma_start(out=yT_sb[:, kt, h1], in_=yT_r[:, kt, h1])
        for kt in range(nkt_qk):
            nc.sync.dma_start(out=xT_sb[:, kt, h1], in_=xT_r[:, kt, h1])
        nc.sync.dma_start(out=wp_a, in_=wpT[0:128, :])
        nc.sync.dma_start(out=wp_b, in_=wpT[128:GW, :])

        # ---------------- projection emitters ------------------------------
        def evac(dst, src, ramp):
            if ramp:
                nc.scalar.copy(dst, src)
            else:
                nc.vector.tensor_copy(dst, src)

        def k_chunk_a(lc, ramp=False):
            sl = slice(lc * 512, (lc + 1) * 512)
            ps = projpool.tile([128, 512], F32, tag="proj")
            for kt in range(nkt_qk):
                nc.tensor.matmul(ps, wk_sb[:, kt, 0:128], yT_sb[:, kt, sl],
                                 start=(kt == 0), stop=(kt == nkt_qk - 1))
            evac(kT_p[:, sl], ps, ramp)

        def k_chunk_b(lc, ramp=False):
            sl = slice(lc * 512, (lc + 1) * 512)
            ps2 = projpool.tile([64, 512], F32, tag="proj")
            for kt in range(nkt_qk):
                nc.tensor.matmul(ps2, wk_sb[:, kt, 128:GW], yT_sb[:, kt, sl],
                                 start=(kt == 0), stop=(kt == nkt_qk - 1))
            evac(kT_2[0:64, sl], ps2, ramp)
            nc.gpsimd.tensor_copy(kT_2[64:128, sl], kT_2[0:64, sl])

        def v_chunk(m, ramp=False):
            ms = slice(m * 128, (m + 1) * 128)
            ps = projpool.tile([128, GW], F32, tag="proj")
            for kt in range(KT):
                nc.tensor.matmul(ps, yT_sb[:, kt, ms], wv_sb[:, kt, :],
                                 start=(kt == 0), stop=(kt == KT - 1))
            evac(v_sb[:, m, :], ps, ramp)

        def q_chunk(lc, pair, ramp=False):
            sl = slice(lc * 512, (lc + 1) * 512)
            if pair:
                ps = projpool.tile([128, 512], F32, tag="proj")
                for kt in range(nkt_qk):
                    nc.tensor.matmul(ps, wq_sb[:, kt, 0:128], xT_sb[:, kt, sl],
                                     start=(kt == 0), stop=(kt == nkt_qk - 1))
                evac(qT_p[:, sl], ps, ramp)
            else:
                ps2 = projpool.tile([64, 512], F32, tag="proj")
                for kt in range(nkt_qk):
                    nc.tensor.matmul(ps2, wq_sb[:, kt, 128:GW], xT_sb[:, kt, sl],
                                     start=(kt == 0), stop=(kt == nkt_qk - 1))
                evac(qT_2[0:64, sl], ps2, ramp)
                nc.gpsimd.tensor_copy(qT_2[64:128, sl], qT_2[0:64, sl])

        def p_tile(lc, o):
            sl = slice(lc * 512, (lc + 1) * 512)
            osl = slice(o * 128, (o + 1) * 128)
            ps = projpool.tile([128, 512], F32, tag="proj")
            nc.tensor.matmul(ps, wp_a[:, osl], on_p[:, sl], start=True, stop=False)
            nc.tensor.matmul(ps, wp_b[:, osl], on_2[:, sl], start=False, stop=True)
            po = ptpool.tile([128, 512], F32, tag="po")
            nc.vector.tensor_copy(po, ps)
            nc.sync.dma_start(out=pT[osl, sl], in_=po)

        # ---------------- denominator chains --------------------------------
        GPS_B0 = (2, 5, 8, 11, 14)
        GPS_STD = (4, 9, 14)

        def make_acc_state(gps_set):
            return {"gps": set(gps_set), "first": [True, True, True], "vn": 0}

        def acc_add(m, acc, pt, st):
            if m in st["gps"]:
                c = 2
                if st["first"][2]:
                    nc.gpsimd.tensor_copy(acc[:, 2, :], pt)
                    st["first"][2] = False
                else:
                    nc.gpsimd.tensor_add(acc[:, 2, :], acc[:, 2, :], pt)
            else:
                c = st["vn"]
                st["vn"] ^= 1
                if st["first"][c]:
                    nc.vector.tensor_copy(acc[:, c, :], pt)
                    st["first"][c] = False
                else:
                    nc.vector.tensor_add(acc[:, c, :], acc[:, c, :], pt)

        def merge_acc(acc):
            nc.vector.tensor_add(acc[:, 0, :], acc[:, 0, :], acc[:, 1, :])
            nc.vector.tensor_add(acc[:, 0, :], acc[:, 0, :], acc[:, 2, :])

        # Normalization of one slot, split into thunks paced into the next
        # slot's m-loop (keeps den matmuls off PE's critical path).
        def norm_thunks(o_ps, acc, dsts):
            """dsts: [(dst_ap, acc_slice, o_ps_slice), ...] (2 entries)."""
            recs = []

            def t0():
                merge_acc(acc)

            def mk_den(i):
                def t():
                    dst, asl, osl = dsts[i]
                    den = projpool.tile([1, 512], F32, tag="proj")
                    nc.tensor.matmul(den, ones_col, acc[:, 0, asl],
                                     start=True, stop=True)
                    recip = rpool.tile([1, 512], F32, tag="recip")
                    nc.vector.reciprocal_approx_fast(out=recip, in_=den)
                    bc = bcpool.tile([64, 512], F32, tag="bc")
                    nc.gpsimd.partition_broadcast(bc, recip)
                    recs.append((dst, osl, bc))
                return t

            def mk_mul(i):
                def t():
                    dst, osl, bc = recs[i]
                    nc.vector.tensor_mul(dst, o_ps[osl], bc)
                return t

            return [t0, mk_den(0), mk_den(1), mk_mul(0), mk_mul(1)]

        # ---------------- attention block emitters ---------------------------
        def slot_a_m(m, sl, s_ps, o_ps, acc, st, first, last):
            ms = slice(m * 128, (m + 1) * 128)
            nc.tensor.matmul(s_ps[:, 0:512], kT_p[0:64, ms], qT_p[0:64, sl],
                             tile_position=(0, 0), start=True, stop=True)
            nc.tensor.matmul(s_ps[:, 512:1024], kT_p[64:128, ms], qT_p[64:128, sl],
                             tile_position=(64, 0), start=True, stop=True)
            pt = ptpool.tile([128, 1024], BF16, tag="pt")
            nc.scalar.activation(pt, s_ps, EXP, scale=SCALE)
            nc.tensor.matmul(o_ps[0:64, :], v_sb[:, m, 0:64], pt[:, 0:512],
                             tile_position=(0, 0), start=first, stop=last)
            nc.tensor.matmul(o_ps[64:128, :], v_sb[:, m, 64:128], pt[:, 512:1024],
                             tile_position=(0, 64), start=first, stop=last)
            acc_add(m, acc, pt, st)

        def slot_b_m(m, sl0, sl1, s_ps, o_ps, acc, st, first, last):
            ms = slice(m * 128, (m + 1) * 128)
            nc.tensor.matmul(s_ps[:, 0:512], kT_2[0:64, ms], qT_2[0:64, sl0],
                             tile_position=(0, 0), start=True, stop=True)
            nc.tensor.matmul(s_ps[:, 512:1024], kT_2[64:128, ms], qT_2[64:128, sl1],
                             tile_position=(64, 0), start=True, stop=True)
            pt = ptpool.tile([128, 1024], BF16, tag="pt")
            nc.scalar.activation(pt, s_ps, EXP, scale=SCALE)
            nc.tensor.matmul(o_ps[0:64, :], v_sb[:, m, 128:GW], pt[:, 0:512],
                             tile_position=(0, 0), start=first, stop=last)
            nc.tensor.matmul(o_ps[64:128, :], v_sb[:, m, 128:GW], pt[:, 512:1024],
                             tile_position=(0, 64), start=first, stop=last)
            acc_add(m, acc, pt, st)

        # ---------------- ramp: minimum work before attention ----------------
        k_chunk_a(0, ramp=True)
        k_chunk_b(0, ramp=True)
        for m in range(4):
            v_chunk(m, ramp=True)
        q_chunk(0, pair=False, ramp=True)
        q_chunk(1, pair=False, ramp=True)

        # ---------------- slot B pair 0 (head 2, l-chunks 0,1) ---------------
        pace_b0 = {
            0: [lambda: k_chunk_a(1), lambda: v_chunk(4)],
            1: [lambda: k_chunk_b(1), lambda: v_chunk(5)],
            2: [lambda: v_chunk(6)],
            3: [lambda: v_chunk(7)],
            4: [lambda: k_chunk_a(2), lambda: v_chunk(8)],
            5: [lambda: k_chunk_b(2), lambda: v_chunk(9)],
            6: [lambda: v_chunk(10)],
            7: [lambda: v_chunk(11)],
            8: [lambda: k_chunk_a(3), lambda: v_chunk(12)],
            9: [lambda: k_chunk_b(3), lambda: v_chunk(13)],
            10: [lambda: v_chunk(14)],
            11: [lambda: v_chunk(15)],
            12: [lambda: q_chunk(2, pair=False)],
            13: [lambda: q_chunk(3, pair=False)],
        }
        o_ps_b0 = opool.tile([128, 512], F32, tag="ops")
        acc_b0 = accpool.tile([128, 3, 1024], BF16, tag="acc")
        st_b0 = make_acc_state(GPS_B0)
        sl0, sl1 = slice(0, 512), slice(512, 1024)
        for m in range(NM):
            s_ps = spool.tile([128, 1024], F32, tag="s")
            slot_b_m(m, sl0, sl1, s_ps, o_ps_b0, acc_b0, st_b0, m == 0, m == NM - 1)
            for th in pace_b0.get(m, ()):
                th()
        pend = norm_thunks(o_ps_b0, acc_b0,
                           [(on_2[:, sl0], slice(0, 512), slice(0, 64)),
                            (on_2[:, sl1], slice(512, 1024), slice(64, 128))])

        # ---------------- slot B pair 1 (head 2, l-chunks 2,3) ---------------
        pace_b1 = {
            0: [pend[0]], 1: [pend[1]], 2: [pend[2]], 3: [pend[3], pend[4]],
            5: [lambda: q_chunk(0, pair=True)],
            8: [lambda: q_chunk(1, pair=True)],
            11: [lambda: q_chunk(2, pair=True)],
        }
        o_ps_b1 = opool.tile([128, 512], F32, tag="ops")
        acc_b1 = accpool.tile([128, 3, 1024], BF16, tag="acc")
        st_b1 = make_acc_state(GPS_STD)
        sl2, sl3 = slice(1024, 1536), slice(1536, 2048)
        for m in range(NM):
            s_ps = spool.tile([128, 1024], F32, tag="s")
            slot_b_m(m, sl2, sl3, s_ps, o_ps_b1, acc_b1, st_b1, m == 0, m == NM - 1)
            for th in pace_b1.get(m, ()):
                th()
        pend = norm_thunks(o_ps_b1, acc_b1,
                           [(on_2[:, sl2], slice(0, 512), slice(0, 64)),
                            (on_2[:, sl3], slice(512, 1024), slice(64, 128))])

        # ---------------- slot A per l-chunk (heads 0,1) ----------------------
        for lc in range(NLC):
            sl = slice(lc * 512, (lc + 1) * 512)
            pace = {0: [pend[0]], 1: [pend[1]], 2: [pend[2]],
                    3: [pend[3], pend[4]]}
            if lc == 0:
                pace[7] = [lambda: q_chunk(3, pair=True)]
            else:
                for o in range(6):
                    pace.setdefault(4 + o, []).append(
                        lambda lc=lc, o=o: p_tile(lc - 1, o))
            o_ps = opool.tile([128, 512], F32, tag="ops")
            acc = accpool.tile([128, 3, 1024], BF16, tag="acc")
            st = make_acc_state(GPS_STD)
            for m in range(NM):
                s_ps = spool.tile([128, 1024], F32, tag="s")
                slot_a_m(m, sl, s_ps, o_ps, acc, st, m == 0, m == NM - 1)
                for th in pace.get(m, ()):
                    th()
            pend = norm_thunks(o_ps, acc,
                               [(on_p[0:64, sl], slice(0, 512), slice(0, 64)),
                                (on_p[64:128, sl], slice(512, 1024), slice(64, 128))])

        # ---------------- tail ------------------------------------------------
        for th in pend:
            th()
        for o in range(6):
            p_tile(NLC - 1, o)

    nc.finalize()
    return nc


def _aug_act(a, aug):
    """[L, D] activations -> [da, L]: transpose (+ ones row + zero pad)."""
    if not aug:
        return np.ascontiguousarray(a.T).astype(bfloat16)
    out = np.zeros((DA, L), dtype=bfloat16)
    out[:D] = a.T.astype(bfloat16)
    out[D] = 1.0
    return out


def _aug_w(w_rows, b_rows, aug):
    """[GW, D] weight rows (+ [GW] bias) -> [da, GW] lhsT."""
    if not aug:
        return np.ascontiguousarray(w_rows.T).astype(bfloat16)
    out = np.zeros((DA, GW), dtype=bfloat16)
    out[:D] = w_rows.T.astype(bfloat16)
    out[D] = b_rows.astype(bfloat16)
    return out


def _make_in_maps(x, y, Wq, bq, Wk, bk, Wv, bv, Wp, bp, aug):
    in_maps = []
    xTs = [_aug_act(x[b], aug) for b in range(B)]
    yTs = [_aug_act(y[b], aug) for b in range(B)]
    for core in range(8):
        b, g = divmod(core, 4)
        rows = slice(g * GW, (g + 1) * GW)
        in_maps.append({
            "xT": xTs[b],
            "yT": yTs[b],
            "wqT": _aug_w(Wq[rows], bq[rows], aug),
            "wkT": _aug_w(Wk[rows], bk[rows], aug),
            "wvT": np.ascontiguousarray(Wv[rows].T).astype(bfloat16),
            "wpT": np.ascontiguousarray(Wp[:, rows].T).astype(bfloat16),
        })
    return in_maps


def _combine(results, Wv, Wp, bp, bv):
    out = np.zeros((B, L, D), dtype=np.float32)
    for core in range(8):
        b = core // 4
        out[b] += results[core]["pT"].T
    out += (Wp @ bv + bp)[None, None, :]
    return out


_NC = {}


def _get_nc(aug=True):
    if aug not in _NC:
        _NC[aug] = _build_program(KTA if aug else KT)
    return _NC[aug]


def run(inputs, trace=False, trace_cores=None, **kwargs):
    aug = bool(np.any(inputs["bq"]) or np.any(inputs["bk"]))
    nc = _get_nc(aug)
    in_maps = _make_in_maps(aug=aug, **inputs)
    res = run_bass_kernel_spmd(
        nc, in_maps, core_ids=list(range(8)), trace=trace,
        trace_cores=trace_cores, **kwargs)
    out = _combine(res.results, inputs["Wv"], inputs["Wp"],
                   inputs["bp"], inputs["bv"])
    return out, res


def kernel(**inputs):
    inputs = {k: np.asarray(v) for k, v in inputs.items()}
    out, _ = run(inputs, trace=False)
    return out


# revision 16
# speedup vs baseline: 1.0515x; 1.0515x over previous
"""Cross-attention Trainium2 kernel (Bass/Tile), 8-core SPMD.

Sharding: 8 cores = 2 (batch) x 4 (head groups of 3 heads).
Each core computes, for its (b, g):
    q^T = Wq_g @ x_b^T          [192, 2048]  (+bq)
    k^T = Wk_g @ y_b^T          [192, 2048]  (+bk)
    v   = y_b @ Wv_g^T          [2048, 192]
    per head: S^T = k_h q_h^T   [2048(m), 2048(l)] tiles in PSUM
              P^T = exp(S^T/8)  (softmax numerator, bf16)
              O^T = v_h^T P^T   (PSUM accumulated over m)
              den = 1^T P^T, O_n^T = O^T * (1/den)  (broadcast via PE)
    partial^T = Wp_g^T^T @ O_n^T  [768, 2048]  -> DRAM fp32
Host: out[b] = sum_g partial_g^T.T + Wp @ bv + bp.

Engine budget per slot of 16 m-blocks (ACT is the floor at ~17.1us):
exp on ACT; den accumulated on 2 Vector chains + 1 GpSimd chain;
PSUM evacuation on Vector (steady) / Scalar (ramp, ACT idle);
K/Q head-2 dup copies on GpSimd (SBUF->SBUF).
"""

import os
import sys
from contextlib import ExitStack

import numpy as np

for _p in ("/opt/trn_rl_repo", "/root/.axon_site/_ro/trn_rl_repo"):
    if os.path.isdir(_p) and _p not in sys.path:
        sys.path.insert(0, _p)

try:  # make trace=True harmless when the env lacks the NTFF hook module
    import antenv.axon_hooks  # noqa: F401
except Exception:
    import types

    _stub = types.ModuleType("antenv.axon_hooks")
    _stub.get_axon_ntff_profile_hook = lambda: None
    _stub.set_axon_ntff_profile_hook = lambda hook: None
    sys.modules["antenv.axon_hooks"] = _stub

import concourse.bass as bass
import concourse.tile as tile
from concourse import bacc as bacc_mod
from concourse import mybir
from concourse.bass_utils import run_bass_kernel_spmd
from ml_dtypes import bfloat16

F32 = mybir.dt.float32
BF16 = mybir.dt.bfloat16
EXP = mybir.ActivationFunctionType.Exp

B = 2
L = 2048          # query length (also key length)
D = 768
HD = 64           # head dim
HPC = 3           # heads per core
GW = HPC * HD     # 192: head-group width
KT = D // 128     # 6 contraction tiles for V projection
KTA = KT + 1      # 7 tiles for Q/K: 7th carries the bias row (exact bias fold)
DA = KTA * 128    # 896: augmented contraction depth
NLC = L // 512    # 4 l-chunks
NM = L // 128     # 16 m-tiles
SCALE = 1.0 / 8.0  # hd ** -0.5


def _build_program(nkt_qk=KTA):
    nc = bacc_mod.Bacc()

    da = nkt_qk * 128
    xT = nc.dram_tensor("xT", [da, L], BF16, kind="ExternalInput")[:, :]
    yT = nc.dram_tensor("yT", [da, L], BF16, kind="ExternalInput")[:, :]
    wqT = nc.dram_tensor("wqT", [da, GW], BF16, kind="ExternalInput")[:, :]
    wkT = nc.dram_tensor("wkT", [da, GW], BF16, kind="ExternalInput")[:, :]
    wvT = nc.dram_tensor("wvT", [D, GW], BF16, kind="ExternalInput")[:, :]
    wpT = nc.dram_tensor("wpT", [GW, D], BF16, kind="ExternalInput")[:, :]
    pT = nc.dram_tensor("pT", [D, L], F32, kind="ExternalOutput")[:, :]

    with tile.TileContext(nc) as tc, ExitStack() as ctx:
        persist = ctx.enter_context(tc.tile_pool(name="persist", bufs=1))
        spool = ctx.enter_context(tc.tile_pool(name="spool", bufs=2, space="PSUM"))
        opool = ctx.enter_context(tc.tile_pool(name="opool", bufs=2, space="PSUM"))
        projpool = ctx.enter_context(tc.tile_pool(name="projpool", bufs=2, space="PSUM"))
        ptpool = ctx.enter_context(tc.tile_pool(name="ptpool", bufs=8))
        accpool = ctx.enter_context(tc.tile_pool(name="accpool", bufs=2))
        rpool = ctx.enter_context(tc.tile_pool(name="rpool", bufs=2))

        # ---------------- persistent SBUF tensors --------------------------
        xT_sb = persist.tile([128, nkt_qk, L], BF16, tag="xT")
        yT_sb = persist.tile([128, nkt_qk, L], BF16, tag="yT")
        wq_sb = persist.tile([128, nkt_qk, GW], BF16, tag="wq")
        wk_sb = persist.tile([128, nkt_qk, GW], BF16, tag="wk")
        wv_sb = persist.tile([128, KT, GW], BF16, tag="wv")
        wp_a = persist.tile([128, D], BF16, tag="wpa")
        wp_b = persist.tile([64, D], BF16, tag="wpb")
        garb = persist.tile([128, 256], BF16, tag="garb")
        ones_col = persist.tile([128, 1], BF16, tag="onesc")
        ones_row = persist.tile([1, 64], F32, tag="onesr")
        dex = persist.tile([1, 8], F32, tag="dex")

        qT_p = persist.tile([128, L], BF16, tag="qTp")   # heads 0,1 stacked
        qT_2 = persist.tile([128, L], BF16, tag="qT2")   # head 2, dup halves
        kT_p = persist.tile([128, L], BF16, tag="kTp")
        kT_2 = persist.tile([128, L], BF16, tag="kT2")   # head 2, dup halves
        v_sb = persist.tile([128, NM, GW], BF16, tag="v")
        on_p = persist.tile([128, L], BF16, tag="onp")   # normalized O^T heads 0,1
        on_2 = persist.tile([64, L], BF16, tag="on2")    # head 2

        # ---------------- warmup: PE p-state + exp table preload -----------
        nc.vector.memset(garb, 0.0)
        nc.vector.memset(ones_col, 1.0)
        nc.vector.memset(ones_row, 1.0)
        nc.vector.memset(dex, 0.0)
        nc.scalar.activation(dex, dex, EXP)  # pull exp table load to t=0
        wps = projpool.tile([1, 256], F32, tag="proj")
        for _ in range(12):
            nc.tensor.matmul(wps, ones_col, garb, start=True, stop=True)

        # ---------------- input DMA (first-needed first) --------------------
        xT_r = xT.rearrange("(kt p) l -> p kt l", p=128)
        yT_r = yT.rearrange("(kt p) l -> p kt l", p=128)
        wk_r = wkT.rearrange("(kt p) g -> p kt g", p=128)
        wq_r = wqT.rearrange("(kt p) g -> p kt g", p=128)
        wv_r = wvT.rearrange("(kt p) g -> p kt g", p=128)
        q0, q1 = slice(0, 512), slice(512, 1024)
        h1 = slice(L // 2, L)
        nc.sync.dma_start(out=wk_sb, in_=wk_r)
        for kt in range(nkt_qk):
            nc.sync.dma_start(out=yT_sb[:, kt, q0], in_=yT_r[:, kt, q0])
        nc.sync.dma_start(out=wv_sb, in_=wv_r)
        nc.sync.dma_start(out=wq_sb, in_=wq_r)
        for kt in range(nkt_qk):
            nc.sync.dma_start(out=yT_sb[:, kt, q1], in_=yT_r[:, kt, q1])
        for kt in range(nkt_qk):
            nc.sync.dma_start(out=xT_sb[:, kt, 0:1024], in_=xT_r[:, kt, 0:1024])
        for kt in range(nkt_qk):
            nc.sync.dma_start(out=yT_sb[:, kt, h1], in_=yT_r[:, kt, h1])
        for kt in range(nkt_qk):
            nc.sync.dma_start(out=xT_sb[:, kt, h1], in_=xT_r[:, kt, h1])
        nc.sync.dma_start(out=wp_a, in_=wpT[0:128, :])
        nc.sync.dma_start(out=wp_b, in_=wpT[128:GW, :])

        # ---------------- projection emitters ------------------------------
        def evac(dst, src, ramp):
            if ramp:
                nc.scalar.copy(dst, src)
            else:
                nc.vector.tensor_copy(dst, src)

        def k_chunk_a(lc, ramp=False):
            sl = slice(lc * 512, (lc + 1) * 512)
            ps = projpool.tile([128, 512], F32, tag="proj")
            for kt in range(nkt_qk):
                nc.tensor.matmul(ps, wk_sb[:, kt, 0:128], yT_sb[:, kt, sl],
                                 start=(kt == 0), stop=(kt == nkt_qk - 1))
            evac(kT_p[:, sl], ps, ramp)

        def k_chunk_b(lc, ramp=False):
            sl = slice(lc * 512, (lc + 1) * 512)
            ps2 = projpool.tile([64, 512], F32, tag="proj")
            for kt in range(nkt_qk):
                nc.tensor.matmul(ps2, wk_sb[:, kt, 128:GW], yT_sb[:, kt, sl],
                                 start=(kt == 0), stop=(kt == nkt_qk - 1))
            evac(kT_2[0:64, sl], ps2, ramp)
            evac(kT_2[64:128, sl], ps2, ramp)

        def v_chunk(m, ramp=False):
            ms = slice(m * 128, (m + 1) * 128)
            ps = projpool.tile([128, GW], F32, tag="proj")
            for kt in range(KT):
                nc.tensor.matmul(ps, yT_sb[:, kt, ms], wv_sb[:, kt, :],
                                 start=(kt == 0), stop=(kt == KT - 1))
            evac(v_sb[:, m, :], ps, ramp)

        def q_chunk(lc, pair, ramp=False):
            sl = slice(lc * 512, (lc + 1) * 512)
            if pair:
                ps = projpool.tile([128, 512], F32, tag="proj")
                for kt in range(nkt_qk):
                    nc.tensor.matmul(ps, wq_sb[:, kt, 0:128], xT_sb[:, kt, sl],
                                     start=(kt == 0), stop=(kt == nkt_qk - 1))
                evac(qT_p[:, sl], ps, ramp)
            else:
                ps2 = projpool.tile([64, 512], F32, tag="proj")
                for kt in range(nkt_qk):
                    nc.tensor.matmul(ps2, wq_sb[:, kt, 128:GW], xT_sb[:, kt, sl],
                                     start=(kt == 0), stop=(kt == nkt_qk - 1))
                evac(qT_2[0:64, sl], ps2, ramp)
                evac(qT_2[64:128, sl], ps2, ramp)

        def p_tile(lc, o):
            sl = slice(lc * 512, (lc + 1) * 512)
            osl = slice(o * 128, (o + 1) * 128)
            ps = projpool.tile([128, 512], F32, tag="proj")
            nc.tensor.matmul(ps, wp_a[:, osl], on_p[:, sl], start=True, stop=False)
            nc.tensor.matmul(ps, wp_b[:, osl], on_2[:, sl], start=False, stop=True)
            po = ptpool.tile([128, 512], F32, tag="po")
            nc.vector.tensor_copy(po, ps)
            nc.sync.dma_start(out=pT[osl, sl], in_=po)

        # ---------------- denominator chains --------------------------------
        GPS_STD = (2, 5, 8, 11, 14)

        def make_acc_state(gps_set):
            return {"gps": set(gps_set), "first": [True, True, True], "vn": 0}

        def acc_add(m, acc, pt, st):
            if m in st["gps"]:
                if st["first"][2]:
                    nc.vector.tensor_copy(acc[:, 2, :], pt)
                    st["first"][2] = False
                else:
                    nc.gpsimd.tensor_add(acc[:, 2, :], acc[:, 2, :], pt)
            else:
                c = st["vn"]
                st["vn"] ^= 1
                if st["first"][c]:
                    nc.vector.tensor_copy(acc[:, c, :], pt)
                    st["first"][c] = False
                else:
                    nc.vector.tensor_add(acc[:, c, :], acc[:, c, :], pt)

        # Normalization of one slot, split into thunks paced into the next
        # slot's m-loop.  den sums the 3 unmerged chains via PE accumulation;
        # the 1->64 partition broadcast of 1/den is also a PE matmul (GpSimd
        # partition_broadcast costs ~1us + forced drains).
        def norm_thunks(o_ps, acc, dsts):
            """dsts: [(dst_ap, acc_slice, o_ps_slice), ...] (2 entries)."""
            recs = []

            def mk_den(i):
                def t():
                    dst, asl, osl = dsts[i]
                    den = projpool.tile([1, 512], F32, tag="proj")
                    for c in range(3):
                        nc.tensor.matmul(den, ones_col, acc[:, c, asl],
                                         start=(c == 0), stop=(c == 2))
                    recip = rpool.tile([1, 512], F32, tag="recip")
                    nc.vector.reciprocal_approx_fast(out=recip, in_=den)
                    bc = rpool.tile([64, 512], F32, tag="bc")
                    nc.gpsimd.partition_broadcast(bc, recip)
                    recs.append((dst, osl, bc))
                return t

            def mk_mul(i):
                def t():
                    dst, osl, bc = recs[i]
                    nc.vector.tensor_mul(dst, o_ps[osl], bc)
                return t

            return [mk_den(0), mk_den(1), mk_mul(0), mk_mul(1)]

        # ---------------- attention block emitters ---------------------------
        def slot_a_m(m, sl, s_ps, o_ps, acc, st, first, last):
            ms = slice(m * 128, (m + 1) * 128)
            nc.tensor.matmul(s_ps[:, 0:512], kT_p[0:64, ms], qT_p[0:64, sl],
                             tile_position=(0, 0), start=True, stop=True)
            nc.tensor.matmul(s_ps[:, 512:1024], kT_p[64:128, ms], qT_p[64:128, sl],
                             tile_position=(64, 0), start=True, stop=True)
            pt = ptpool.tile([128, 1024], BF16, tag="pt")
            nc.scalar.activation(pt, s_ps, EXP, scale=SCALE)
            nc.tensor.matmul(o_ps[0:64, :], v_sb[:, m, 0:64], pt[:, 0:512],
                             tile_position=(0, 0), start=first, stop=last)
            nc.tensor.matmul(o_ps[64:128, :], v_sb[:, m, 64:128], pt[:, 512:1024],
                             tile_position=(0, 64), start=first, stop=last)
            acc_add(m, acc, pt, st)

        def slot_b_m(m, sl0, sl1, s_ps, o_ps, acc, st, first, last):
            ms = slice(m * 128, (m + 1) * 128)
            nc.tensor.matmul(s_ps[:, 0:512], kT_2[0:64, ms], qT_2[0:64, sl0],
                             tile_position=(0, 0), start=True, stop=True)
            nc.tensor.matmul(s_ps[:, 512:1024], kT_2[64:128, ms], qT_2[64:128, sl1],
                             tile_position=(64, 0), start=True, stop=True)
            pt = ptpool.tile([128, 1024], BF16, tag="pt")
            nc.scalar.activation(pt, s_ps, EXP, scale=SCALE)
            nc.tensor.matmul(o_ps[0:64, :], v_sb[:, m, 128:GW], pt[:, 0:512],
                             tile_position=(0, 0), start=first, stop=last)
            nc.tensor.matmul(o_ps[64:128, :], v_sb[:, m, 128:GW], pt[:, 512:1024],
                             tile_position=(0, 64), start=first, stop=last)
            acc_add(m, acc, pt, st)

        # ---------------- ramp: minimum work before attention ----------------
        k_chunk_a(0, ramp=True)
        k_chunk_b(0, ramp=True)
        for m in range(4):
            v_chunk(m, ramp=True)
        q_chunk(0, pair=False, ramp=True)
        q_chunk(1, pair=False, ramp=True)

        # ---------------- slot B pair 0 (head 2, l-chunks 0,1) ---------------
        pace_b0 = {
            0: [lambda: k_chunk_a(1), lambda: v_chunk(4)],
            1: [lambda: k_chunk_b(1), lambda: v_chunk(5)],
            2: [lambda: v_chunk(6)],
            3: [lambda: v_chunk(7)],
            4: [lambda: k_chunk_a(2), lambda: v_chunk(8)],
            5: [lambda: k_chunk_b(2), lambda: v_chunk(9)],
            6: [lambda: v_chunk(10)],
            7: [lambda: v_chunk(11)],
            8: [lambda: k_chunk_a(3), lambda: v_chunk(12)],
            9: [lambda: k_chunk_b(3), lambda: v_chunk(13)],
            10: [lambda: v_chunk(14)],
            11: [lambda: v_chunk(15)],
            12: [lambda: q_chunk(2, pair=False)],
            13: [lambda: q_chunk(3, pair=False)],
        }
        o_ps_b0 = opool.tile([128, 512], F32, tag="ops")
        acc_b0 = accpool.tile([128, 3, 1024], BF16, tag="acc")
        st_b0 = make_acc_state(GPS_STD)
        sl0, sl1 = slice(0, 512), slice(512, 1024)
        for m in range(NM):
            s_ps = spool.tile([128, 1024], F32, tag="s")
            slot_b_m(m, sl0, sl1, s_ps, o_ps_b0, acc_b0, st_b0, m == 0, m == NM - 1)
            for th in pace_b0.get(m, ()):
                th()
        pend = norm_thunks(o_ps_b0, acc_b0,
                           [(on_2[:, sl0], slice(0, 512), slice(0, 64)),
                            (on_2[:, sl1], slice(512, 1024), slice(64, 128))])

        # ---------------- slot B pair 1 (head 2, l-chunks 2,3) ---------------
        pace_b1 = {
            0: [pend[0]], 1: [pend[1]], 2: [pend[2]], 3: [pend[3]],
            5: [lambda: q_chunk(0, pair=True)],
            8: [lambda: q_chunk(1, pair=True)],
            11: [lambda: q_chunk(2, pair=True)],
        }
        o_ps_b1 = opool.tile([128, 512], F32, tag="ops")
        acc_b1 = accpool.tile([128, 3, 1024], BF16, tag="acc")
        st_b1 = make_acc_state(GPS_STD)
        sl2, sl3 = slice(1024, 1536), slice(1536, 2048)
        for m in range(NM):
            s_ps = spool.tile([128, 1024], F32, tag="s")
            slot_b_m(m, sl2, sl3, s_ps, o_ps_b1, acc_b1, st_b1, m == 0, m == NM - 1)
            for th in pace_b1.get(m, ()):
                th()
        pend = norm_thunks(o_ps_b1, acc_b1,
                           [(on_2[:, sl2], slice(0, 512), slice(0, 64)),
                            (on_2[:, sl3], slice(512, 1024), slice(64, 128))])

        # ---------------- slot A per l-chunk (heads 0,1) ----------------------
        for lc in range(NLC):
            sl = slice(lc * 512, (lc + 1) * 512)
            pace = {0: [pend[0]], 1: [pend[1]], 2: [pend[2]], 3: [pend[3]]}
            if lc == 0:
                pace[7] = [lambda: q_chunk(3, pair=True)]
            else:
                for o in range(6):
                    pace.setdefault(4 + o, []).append(
                        lambda lc=lc, o=o: p_tile(lc - 1, o))
            o_ps = opool.tile([128, 512], F32, tag="ops")
            acc = accpool.tile([128, 3, 1024], BF16, tag="acc")
            st = make_acc_state(GPS_STD)
            for m in range(NM):
                s_ps = spool.tile([128, 1024], F32, tag="s")
                slot_a_m(m, sl, s_ps, o_ps, acc, st, m == 0, m == NM - 1)
                for th in pace.get(m, ()):
                    th()
            pend = norm_thunks(o_ps, acc,
                               [(on_p[0:64, sl], slice(0, 512), slice(0, 64)),
                                (on_p[64:128, sl], slice(512, 1024), slice(64, 128))])

        # ---------------- tail ------------------------------------------------
        for th in pend:
            th()
        for o in range(6):
            p_tile(NLC - 1, o)

    nc.finalize()
    return nc


def _aug_act(a, aug):
    """[L, D] activations -> [da, L]: transpose (+ ones row + zero pad)."""
    if not aug:
        return np.ascontiguousarray(a.T).astype(bfloat16)
    out = np.zeros((DA, L), dtype=bfloat16)
    out[:D] = a.T.astype(bfloat16)
    out[D] = 1.0
    return out


def _aug_w(w_rows, b_rows, aug):
    """[GW, D] weight rows (+ [GW] bias) -> [da, GW] lhsT."""
    if not aug:
        return np.ascontiguousarray(w_rows.T).astype(bfloat16)
    out = np.zeros((DA, GW), dtype=bfloat16)
    out[:D] = w_rows.T.astype(bfloat16)
    out[D] = b_rows.astype(bfloat16)
    return out


def _make_in_maps(x, y, Wq, bq, Wk, bk, Wv, bv, Wp, bp, aug):
    in_maps = []
    xTs = [_aug_act(x[b], aug) for b in range(B)]
    yTs = [_aug_act(y[b], aug) for b in range(B)]
    for core in range(8):
        b, g = divmod(core, 4)
        rows = slice(g * GW, (g + 1) * GW)
        in_maps.append({
            "xT": xTs[b],
            "yT": yTs[b],
            "wqT": _aug_w(Wq[rows], bq[rows], aug),
            "wkT": _aug_w(Wk[rows], bk[rows], aug),
            "wvT": np.ascontiguousarray(Wv[rows].T).astype(bfloat16),
            "wpT": np.ascontiguousarray(Wp[:, rows].T).astype(bfloat16),
        })
    return in_maps


def _combine(results, Wv, Wp, bp, bv):
    out = np.zeros((B, L, D), dtype=np.float32)
    for core in range(8):
        b = core // 4
        out[b] += results[core]["pT"].T
    out += (Wp @ bv + bp)[None, None, :]
    return out


_NC = {}


def _get_nc(aug=True):
    if aug not in _NC:
        _NC[aug] = _build_program(KTA if aug else KT)
    return _NC[aug]


def run(inputs, trace=False, trace_cores=None, **kwargs):
    aug = bool(np.any(inputs["bq"]) or np.any(inputs["bk"]))
    nc = _get_nc(aug)
    in_maps = _make_in_maps(aug=aug, **inputs)
    res = run_bass_kernel_spmd(
        nc, in_maps, core_ids=list(range(8)), trace=trace,
        trace_cores=trace_cores, **kwargs)
    out = _combine(res.results, inputs["Wv"], inputs["Wp"],
                   inputs["bp"], inputs["bv"])
    return out, res


def kernel(**inputs):
    inputs = {k: np.asarray(v) for k, v in inputs.items()}
    out, _ = run(inputs, trace=False)
    return out


# revision 21
# speedup vs baseline: 1.0999x; 1.0460x over previous
"""Cross-attention Trainium2 kernel (Bass/Tile), 8-core SPMD.

Sharding: 8 cores = 2 (batch) x 4 (head groups of 3 heads).
Each core computes, for its (b, g):
    q^T = Wq_g @ x_b^T          [192, 2048]  (+bq)
    k^T = Wk_g @ y_b^T          [192, 2048]  (+bk)
    v   = y_b @ Wv_g^T          [2048, 192]
    per head: S^T = k_h q_h^T   [2048(m), 2048(l)] tiles in PSUM
              P^T = exp(S^T/8)  (softmax numerator, bf16)
              O^T = v_h^T P^T   (PSUM accumulated over m)
              den = 1^T P^T, O_n^T = O^T * (1/den)  (broadcast via PE)
    partial^T = Wp_g^T^T @ O_n^T  [768, 2048]  -> DRAM fp32
Host: out[b] = sum_g partial_g^T.T + Wp @ bv + bp.

Engine budget per slot of 16 m-blocks (ACT is the floor at ~17.1us):
exp on ACT; den accumulated on 2 Vector chains + 1 GpSimd chain;
PSUM evacuation on Vector (steady) / Scalar (ramp, ACT idle);
K/Q head-2 dup copies on GpSimd (SBUF->SBUF).
"""

import os
import sys
from contextlib import ExitStack

import numpy as np

for _p in ("/opt/trn_rl_repo", "/root/.axon_site/_ro/trn_rl_repo"):
    if os.path.isdir(_p) and _p not in sys.path:
        sys.path.insert(0, _p)

try:  # make trace=True harmless when the env lacks the NTFF hook module
    import antenv.axon_hooks  # noqa: F401
except Exception:
    import types

    _stub = types.ModuleType("antenv.axon_hooks")
    _stub.get_axon_ntff_profile_hook = lambda: None
    _stub.set_axon_ntff_profile_hook = lambda hook: None
    sys.modules["antenv.axon_hooks"] = _stub

import concourse.bass as bass
import concourse.tile as tile
from concourse import bacc as bacc_mod
from concourse import mybir
from concourse.bass_utils import run_bass_kernel_spmd
from ml_dtypes import bfloat16

F32 = mybir.dt.float32
BF16 = mybir.dt.bfloat16
EXP = mybir.ActivationFunctionType.Exp

B = 2
L = 2048          # query length (also key length)
D = 768
HD = 64           # head dim
HPC = 3           # heads per core
GW = HPC * HD     # 192: head-group width
KT = D // 128     # 6 contraction tiles for V projection
KTA = KT + 1      # 7 tiles for Q/K: 7th carries the bias row (exact bias fold)
DA = KTA * 128    # 896: augmented contraction depth
NLC = L // 512    # 4 l-chunks
NM = L // 128     # 16 m-tiles
SCALE = 1.0 / 8.0  # hd ** -0.5


def _build_program(nkt_qk=KTA):
    nc = bacc_mod.Bacc()

    da = nkt_qk * 128
    xT = nc.dram_tensor("xT", [da, L], BF16, kind="ExternalInput")[:, :]
    yT = nc.dram_tensor("yT", [da, L], BF16, kind="ExternalInput")[:, :]
    wqT = nc.dram_tensor("wqT", [da, GW], BF16, kind="ExternalInput")[:, :]
    wkT = nc.dram_tensor("wkT", [da, GW], BF16, kind="ExternalInput")[:, :]
    wvT = nc.dram_tensor("wvT", [D, GW], BF16, kind="ExternalInput")[:, :]
    wpT = nc.dram_tensor("wpT", [GW, D], BF16, kind="ExternalInput")[:, :]
    pT = nc.dram_tensor("pT", [D, L], F32, kind="ExternalOutput")[:, :]

    with tile.TileContext(nc) as tc, ExitStack() as ctx:
        persist = ctx.enter_context(tc.tile_pool(name="persist", bufs=1))
        spool = ctx.enter_context(tc.tile_pool(name="spool", bufs=2, space="PSUM"))
        opool = ctx.enter_context(tc.tile_pool(name="opool", bufs=2, space="PSUM"))
        projpool = ctx.enter_context(tc.tile_pool(name="projpool", bufs=2, space="PSUM"))
        ptpool = ctx.enter_context(tc.tile_pool(name="ptpool", bufs=8))
        accpool = ctx.enter_context(tc.tile_pool(name="accpool", bufs=2))
        rpool = ctx.enter_context(tc.tile_pool(name="rpool", bufs=2))

        # ---------------- persistent SBUF tensors --------------------------
        xT_sb = persist.tile([128, nkt_qk, L], BF16, tag="xT")
        yT_sb = persist.tile([128, nkt_qk, L], BF16, tag="yT")
        wq_sb = persist.tile([128, nkt_qk, GW], BF16, tag="wq")
        wk_sb = persist.tile([128, nkt_qk, GW], BF16, tag="wk")
        wv_sb = persist.tile([128, KT, GW], BF16, tag="wv")
        wp_a = persist.tile([128, D], BF16, tag="wpa")
        wp_bd = persist.tile([128, D], BF16, tag="wpbd")
        garb = persist.tile([128, 256], BF16, tag="garb")
        ones_col = persist.tile([128, 1], BF16, tag="onesc")
        ones_row = persist.tile([1, 64], F32, tag="onesr")
        dex = persist.tile([1, 8], F32, tag="dex")

        qT_p = persist.tile([128, L], BF16, tag="qTp")   # heads 0,1 stacked
        qT_2 = persist.tile([128, L], BF16, tag="qT2")   # head 2, dup halves
        kT_p = persist.tile([128, L], BF16, tag="kTp")
        kT_2 = persist.tile([128, L], BF16, tag="kT2")   # head 2, dup halves
        v_sb = persist.tile([128, NM, GW], BF16, tag="v")
        on_p = persist.tile([128, L], BF16, tag="onp")   # normalized O^T heads 0,1
        on_2 = persist.tile([128, L], BF16, tag="on2")   # head 2, dup halves

        # ---------------- warmup: PE p-state + exp table preload -----------
        nc.vector.memset(garb, 0.0)
        nc.vector.memset(ones_col, 1.0)
        nc.vector.memset(ones_row, 1.0)
        nc.vector.memset(dex, 0.0)
        nc.scalar.activation(dex, dex, EXP)  # pull exp table load to t=0
        wps = projpool.tile([1, 256], F32, tag="proj")
        for _ in range(12):
            nc.tensor.matmul(wps, ones_col, garb, start=True, stop=True)

        # ---------------- input DMA (first-needed first) --------------------
        xT_r = xT.rearrange("(kt p) l -> p kt l", p=128)
        yT_r = yT.rearrange("(kt p) l -> p kt l", p=128)
        wk_r = wkT.rearrange("(kt p) g -> p kt g", p=128)
        wq_r = wqT.rearrange("(kt p) g -> p kt g", p=128)
        wv_r = wvT.rearrange("(kt p) g -> p kt g", p=128)
        q0, q1 = slice(0, 512), slice(512, 1024)
        h1 = slice(L // 2, L)
        nc.sync.dma_start(out=wk_sb, in_=wk_r)
        for kt in range(nkt_qk):
            nc.sync.dma_start(out=yT_sb[:, kt, q0], in_=yT_r[:, kt, q0])
        nc.sync.dma_start(out=wv_sb, in_=wv_r)
        nc.sync.dma_start(out=wq_sb, in_=wq_r)
        for kt in range(nkt_qk):
            nc.sync.dma_start(out=yT_sb[:, kt, q1], in_=yT_r[:, kt, q1])
        for kt in range(nkt_qk):
            nc.sync.dma_start(out=xT_sb[:, kt, 0:1024], in_=xT_r[:, kt, 0:1024])
        for kt in range(nkt_qk):
            nc.sync.dma_start(out=yT_sb[:, kt, h1], in_=yT_r[:, kt, h1])
        for kt in range(nkt_qk):
            nc.sync.dma_start(out=xT_sb[:, kt, h1], in_=xT_r[:, kt, h1])
        nc.sync.dma_start(out=wp_a, in_=wpT[0:128, :])
        nc.sync.dma_start(out=wp_bd[0:64, :], in_=wpT[128:GW, :])
        nc.sync.dma_start(out=wp_bd[64:128, :], in_=wpT[128:GW, :])

        # ---------------- projection emitters ------------------------------
        def evac(dst, src, on_scalar):
            if on_scalar:
                nc.scalar.copy(dst, src)
            else:
                nc.vector.tensor_copy(dst, src)

        def k_chunk_a(lc, on_scalar=False):
            sl = slice(lc * 512, (lc + 1) * 512)
            ps = projpool.tile([128, 512], F32, tag="proj")
            for kt in range(nkt_qk):
                nc.tensor.matmul(ps, wk_sb[:, kt, 0:128], yT_sb[:, kt, sl],
                                 start=(kt == 0), stop=(kt == nkt_qk - 1))
            evac(kT_p[:, sl], ps, on_scalar)

        def kq2_pair(lck, lcq, on_scalar=False):
            """col-paired 64-out projections: k head2 chunk lck on cols 0:63,
            q head2 chunk lcq on cols 64:127 -- streams run concurrently."""
            slk = slice(lck * 512, (lck + 1) * 512)
            slq = slice(lcq * 512, (lcq + 1) * 512)
            ps = projpool.tile([128, 512], F32, tag="proj")
            for kt in range(nkt_qk):
                nc.tensor.matmul(ps[0:64, :], wk_sb[:, kt, 128:GW],
                                 yT_sb[:, kt, slk], tile_position=(0, 0),
                                 start=(kt == 0), stop=(kt == nkt_qk - 1))
                nc.tensor.matmul(ps[64:128, :], wq_sb[:, kt, 128:GW],
                                 xT_sb[:, kt, slq], tile_position=(0, 64),
                                 start=(kt == 0), stop=(kt == nkt_qk - 1))
            evac(kT_2[0:64, slk], ps[0:64, :], on_scalar)
            evac(kT_2[64:128, slk], ps[0:64, :], on_scalar)
            evac(qT_2[0:64, slq], ps[64:128, :], on_scalar)
            evac(qT_2[64:128, slq], ps[64:128, :], on_scalar)

        def k_chunk_b(lc, on_scalar=False):
            sl = slice(lc * 512, (lc + 1) * 512)
            ps2 = projpool.tile([64, 512], F32, tag="proj")
            for kt in range(nkt_qk):
                nc.tensor.matmul(ps2, wk_sb[:, kt, 128:GW], yT_sb[:, kt, sl],
                                 start=(kt == 0), stop=(kt == nkt_qk - 1))
            evac(kT_2[0:64, sl], ps2, on_scalar)
            evac(kT_2[64:128, sl], ps2, on_scalar)

        def v_chunk(m, on_scalar=False):
            ms = slice(m * 128, (m + 1) * 128)
            ps = projpool.tile([128, GW], F32, tag="proj")
            for kt in range(KT):
                nc.tensor.matmul(ps, yT_sb[:, kt, ms], wv_sb[:, kt, :],
                                 start=(kt == 0), stop=(kt == KT - 1))
            evac(v_sb[:, m, :], ps, on_scalar)

        def q_chunk2(lc, on_scalar=False):
            sl = slice(lc * 512, (lc + 1) * 512)
            ps2 = projpool.tile([64, 512], F32, tag="proj")
            for kt in range(nkt_qk):
                nc.tensor.matmul(ps2, wq_sb[:, kt, 128:GW], xT_sb[:, kt, sl],
                                 start=(kt == 0), stop=(kt == nkt_qk - 1))
            evac(qT_2[0:64, sl], ps2, on_scalar)
            evac(qT_2[64:128, sl], ps2, on_scalar)

        def q_chunk_p(lc, on_scalar=False):
            sl = slice(lc * 512, (lc + 1) * 512)
            ps = projpool.tile([128, 512], F32, tag="proj")
            for kt in range(nkt_qk):
                nc.tensor.matmul(ps, wq_sb[:, kt, 0:128], xT_sb[:, kt, sl],
                                 start=(kt == 0), stop=(kt == nkt_qk - 1))
            evac(qT_p[:, sl], ps, on_scalar)

        def p_pair(lc, o):
            """two output o-tiles; the 64-deep wp_b passes are row-paired."""
            sl = slice(lc * 512, (lc + 1) * 512)
            osl0 = slice(o * 128, (o + 1) * 128)
            osl1 = slice((o + 1) * 128, (o + 2) * 128)
            ps0 = projpool.tile([128, 512], F32, tag="proj")
            ps1 = projpool.tile([128, 512], F32, tag="proj")
            nc.tensor.matmul(ps0, wp_a[:, osl0], on_p[:, sl], start=True, stop=False)
            nc.tensor.matmul(ps1, wp_a[:, osl1], on_p[:, sl], start=True, stop=False)
            nc.tensor.matmul(ps0, wp_bd[0:64, osl0], on_2[0:64, sl],
                             tile_position=(0, 0), start=False, stop=True)
            nc.tensor.matmul(ps1, wp_bd[64:128, osl1], on_2[64:128, sl],
                             tile_position=(64, 0), start=False, stop=True)
            po0 = ptpool.tile([128, 512], F32, tag="po")
            nc.vector.tensor_copy(po0, ps0)
            nc.sync.dma_start(out=pT[osl0, sl], in_=po0)
            po1 = ptpool.tile([128, 512], F32, tag="po")
            nc.vector.tensor_copy(po1, ps1)
            nc.sync.dma_start(out=pT[osl1, sl], in_=po1)

        # ---------------- denominator chains --------------------------------
        GPS_STD = (2, 5, 8, 11, 14)

        def make_acc_state(gps_set):
            return {"gps": set(gps_set), "first": [True, True, True], "vn": 0}

        def acc_add(m, acc, pt, st):
            if m in st["gps"]:
                if st["first"][2]:
                    nc.vector.tensor_copy(acc[:, 2, :], pt)
                    st["first"][2] = False
                else:
                    nc.gpsimd.tensor_add(acc[:, 2, :], acc[:, 2, :], pt)
            else:
                c = st["vn"]
                st["vn"] ^= 1
                if st["first"][c]:
                    nc.vector.tensor_copy(acc[:, c, :], pt)
                    st["first"][c] = False
                else:
                    nc.vector.tensor_add(acc[:, c, :], acc[:, c, :], pt)

        # Normalization of one slot, split into thunks paced into the next
        # slot's m-loop.  The two 512-l denominators are col-paired into one
        # PSUM tile (rows 0 and 64); each sums the 3 unmerged acc chains by
        # PE accumulation.
        def norm_thunks(o_ps, acc, dsts):
            """dsts: [(list_of_dst_aps, o_ps_slice), ...] (2 entries);
            entry i covers acc free-cols i*512:(i+1)*512."""
            bcs = []

            def t_den():
                for i in range(2):
                    den = projpool.tile([1, 512], F32, tag="proj")
                    for c in range(3):
                        nc.tensor.matmul(den, ones_col,
                                         acc[:, c, 512 * i:512 * (i + 1)],
                                         start=(c == 0), stop=(c == 2))
                    recip = rpool.tile([1, 512], F32, tag="recip")
                    nc.vector.reciprocal_approx_fast(out=recip, in_=den)
                    bc = rpool.tile([64, 512], F32, tag="bc")
                    nc.gpsimd.partition_broadcast(bc, recip)
                    bcs.append(bc)

            def mk_mul(i):
                def t():
                    dst_list, osl = dsts[i]
                    for dst in dst_list:
                        nc.vector.tensor_mul(dst, o_ps[osl], bcs[i])
                return t

            return [t_den, mk_mul(0), mk_mul(1)]

        # ---------------- attention block emitters ---------------------------
        def slot_a_m(m, sl, s_ps, o_ps, acc, st, first, last):
            ms = slice(m * 128, (m + 1) * 128)
            nc.tensor.matmul(s_ps[:, 0:512], kT_p[0:64, ms], qT_p[0:64, sl],
                             tile_position=(0, 0), start=True, stop=True)
            nc.tensor.matmul(s_ps[:, 512:1024], kT_p[64:128, ms], qT_p[64:128, sl],
                             tile_position=(64, 0), start=True, stop=True)
            pt = ptpool.tile([128, 1024], BF16, tag="pt")
            nc.scalar.activation(pt, s_ps, EXP, scale=SCALE)
            nc.tensor.matmul(o_ps[0:64, :], v_sb[:, m, 0:64], pt[:, 0:512],
                             tile_position=(0, 0), start=first, stop=last)
            nc.tensor.matmul(o_ps[64:128, :], v_sb[:, m, 64:128], pt[:, 512:1024],
                             tile_position=(0, 64), start=first, stop=last)
            acc_add(m, acc, pt, st)

        def slot_b_m(m, sl0, sl1, s_ps, o_ps, acc, st, first, last):
            ms = slice(m * 128, (m + 1) * 128)
            nc.tensor.matmul(s_ps[:, 0:512], kT_2[0:64, ms], qT_2[0:64, sl0],
                             tile_position=(0, 0), start=True, stop=True)
            nc.tensor.matmul(s_ps[:, 512:1024], kT_2[64:128, ms], qT_2[64:128, sl1],
                             tile_position=(64, 0), start=True, stop=True)
            pt = ptpool.tile([128, 1024], BF16, tag="pt")
            nc.scalar.activation(pt, s_ps, EXP, scale=SCALE)
            nc.tensor.matmul(o_ps[0:64, :], v_sb[:, m, 128:GW], pt[:, 0:512],
                             tile_position=(0, 0), start=first, stop=last)
            nc.tensor.matmul(o_ps[64:128, :], v_sb[:, m, 128:GW], pt[:, 512:1024],
                             tile_position=(0, 64), start=first, stop=last)
            acc_add(m, acc, pt, st)

        # ---------------- ramp: minimum work before attention ----------------
        k_chunk_a(0, on_scalar=True)
        kq2_pair(0, 0, on_scalar=True)     # k head2 lc0 + q head2 lc0
        q_chunk2(1, on_scalar=True)        # q head2 lc1
        for m in range(4):
            v_chunk(m)                     # vector evac; ACT is busy above

        # ---------------- slot B pair 0 (head 2, l-chunks 0,1) ---------------
        pace_b0 = {
            0: [lambda: k_chunk_a(1), lambda: v_chunk(4)],
            1: [lambda: kq2_pair(1, 2, on_scalar=True)],
            2: [lambda: v_chunk(5)],
            3: [lambda: v_chunk(6)],
            4: [lambda: v_chunk(7)],
            5: [lambda: kq2_pair(2, 3, on_scalar=True)],
            6: [lambda: v_chunk(8)],
            7: [lambda: v_chunk(9)],
            8: [lambda: k_chunk_b(3, on_scalar=True)],
            9: [lambda: v_chunk(10), lambda: v_chunk(11)],
            10: [lambda: v_chunk(12), lambda: v_chunk(13)],
            11: [lambda: v_chunk(14), lambda: v_chunk(15)],
        }
        o_ps_b0 = opool.tile([128, 512], F32, tag="ops")
        acc_b0 = accpool.tile([128, 3, 1024], BF16, tag="acc")
        st_b0 = make_acc_state(GPS_STD)
        sl0, sl1 = slice(0, 512), slice(512, 1024)
        for m in range(NM):
            s_ps = spool.tile([128, 1024], F32, tag="s")
            slot_b_m(m, sl0, sl1, s_ps, o_ps_b0, acc_b0, st_b0, m == 0, m == NM - 1)
            for th in pace_b0.get(m, ()):
                th()
        pend = norm_thunks(o_ps_b0, acc_b0,
                           [([on_2[0:64, sl0], on_2[64:128, sl0]], slice(0, 64)),
                            ([on_2[0:64, sl1], on_2[64:128, sl1]], slice(64, 128))])

        # ---------------- slot B pair 1 (head 2, l-chunks 2,3) ---------------
        pace_b1 = {
            0: [pend[0]], 1: [pend[1]], 2: [pend[2]],
            4: [lambda: k_chunk_a(2)],
            8: [lambda: k_chunk_a(3)],
            12: [lambda: q_chunk_p(0)],
        }
        o_ps_b1 = opool.tile([128, 512], F32, tag="ops")
        acc_b1 = accpool.tile([128, 3, 1024], BF16, tag="acc")
        st_b1 = make_acc_state(GPS_STD)
        sl2, sl3 = slice(1024, 1536), slice(1536, 2048)
        for m in range(NM):
            s_ps = spool.tile([128, 1024], F32, tag="s")
            slot_b_m(m, sl2, sl3, s_ps, o_ps_b1, acc_b1, st_b1, m == 0, m == NM - 1)
            for th in pace_b1.get(m, ()):
                th()
        pend = norm_thunks(o_ps_b1, acc_b1,
                           [([on_2[0:64, sl2], on_2[64:128, sl2]], slice(0, 64)),
                            ([on_2[0:64, sl3], on_2[64:128, sl3]], slice(64, 128))])

        # ---------------- slot A per l-chunk (heads 0,1) ----------------------
        for lc in range(NLC):
            sl = slice(lc * 512, (lc + 1) * 512)
            pace = {0: [pend[0]], 1: [pend[1]], 2: [pend[2]]}
            if lc < NLC - 1:
                pace[4] = [lambda lc=lc: q_chunk_p(lc + 1)]
            if lc > 0:
                for j, o in enumerate((0, 2, 4)):
                    pace.setdefault(7 + 3 * j, []).append(
                        lambda lc=lc, o=o: p_pair(lc - 1, o))
            o_ps = opool.tile([128, 512], F32, tag="ops")
            acc = accpool.tile([128, 3, 1024], BF16, tag="acc")
            st = make_acc_state(GPS_STD if lc < NLC - 1 else (14,))
            for m in range(NM):
                s_ps = spool.tile([128, 1024], F32, tag="s")
                slot_a_m(m, sl, s_ps, o_ps, acc, st, m == 0, m == NM - 1)
                for th in pace.get(m, ()):
                    th()
            pend = norm_thunks(o_ps, acc,
                               [([on_p[0:64, sl]], slice(0, 64)),
                                ([on_p[64:128, sl]], slice(64, 128))])

        # ---------------- tail ------------------------------------------------
        for th in pend:
            th()
        for o in (0, 2, 4):
            p_pair(NLC - 1, o)

    nc.finalize()
    return nc


def _aug_act(a, aug):
    """[L, D] activations -> [da, L]: transpose (+ ones row + zero pad)."""
    if not aug:
        return np.ascontiguousarray(a.T).astype(bfloat16)
    out = np.zeros((DA, L), dtype=bfloat16)
    out[:D] = a.T.astype(bfloat16)
    out[D] = 1.0
    return out


def _aug_w(w_rows, b_rows, aug):
    """[GW, D] weight rows (+ [GW] bias) -> [da, GW] lhsT."""
    if not aug:
        return np.ascontiguousarray(w_rows.T).astype(bfloat16)
    out = np.zeros((DA, GW), dtype=bfloat16)
    out[:D] = w_rows.T.astype(bfloat16)
    out[D] = b_rows.astype(bfloat16)
    return out


def _make_in_maps(x, y, Wq, bq, Wk, bk, Wv, bv, Wp, bp, aug):
    in_maps = []
    xTs = [_aug_act(x[b], aug) for b in range(B)]
    yTs = [_aug_act(y[b], aug) for b in range(B)]
    for core in range(8):
        b, g = divmod(core, 4)
        rows = slice(g * GW, (g + 1) * GW)
        in_maps.append({
            "xT": xTs[b],
            "yT": yTs[b],
            "wqT": _aug_w(Wq[rows], bq[rows], aug),
            "wkT": _aug_w(Wk[rows], bk[rows], aug),
            "wvT": np.ascontiguousarray(Wv[rows].T).astype(bfloat16),
            "wpT": np.ascontiguousarray(Wp[:, rows].T).astype(bfloat16),
        })
    return in_maps


def _combine(results, Wv, Wp, bp, bv):
    out = np.zeros((B, L, D), dtype=np.float32)
    for core in range(8):
        b = core // 4
        out[b] += results[core]["pT"].T
    out += (Wp @ bv + bp)[None, None, :]
    return out


_NC = {}


def _get_nc(aug=True):
    if aug not in _NC:
        _NC[aug] = _build_program(KTA if aug else KT)
    return _NC[aug]


def run(inputs, trace=False, trace_cores=None, **kwargs):
    aug = bool(np.any(inputs["bq"]) or np.any(inputs["bk"]))
    nc = _get_nc(aug)
    in_maps = _make_in_maps(aug=aug, **inputs)
    res = run_bass_kernel_spmd(
        nc, in_maps, core_ids=list(range(8)), trace=trace,
        trace_cores=trace_cores, **kwargs)
    out = _combine(res.results, inputs["Wv"], inputs["Wp"],
                   inputs["bp"], inputs["bv"])
    return out, res


def kernel(**inputs):
    inputs = {k: np.asarray(v) for k, v in inputs.items()}
    out, _ = run(inputs, trace=False)
    return out
